# revision 1
# baseline (speedup 1.0000x reference)
"""Trainium2 Bass kernel for gaussian-weighted box-feature scatter (pooling).

Math (from the reference):
    out[c,h,w] = (1/N) * sum_n box_feats[c,n] * gmaps[n,h,w]
with gmaps separable:
    gmaps[n,h,w] = exp(-(h - x1[n])^2 / (2 s_n^2)) * exp(-w^2 / (2 s_n^2))
                 = gy[n,h] * gx[n,w]

Host (tiny, O(N*C + N*(H+W))): box corner math, one bilinear sample per box
(box_feats [C,N]), and the two 1-D gaussian profiles gy [N,H], gx [N,W].

Device (heavy, O(C*H*W) = 268 MB of output): rank-N reconstruction
    out[c,h,w] = sum_n (A[c,n]*gy[n,h]) * gx[n,w],   A = box_feats/N
done as per-h matmuls on the PE: lhsT = B_h[n,c] = A_T[n,c]*gy[n,h] (DVE
tensor_scalar), rhs = gx [N,W], accumulating K=N=20 in one shot into PSUM,
then PSUM->SBUF copy and large staged DMA writes to HBM.

Sharding: H split across the 8 cores (64 rows each) — fully local, no
communication. Per-core HBM traffic is dominated by the 33.5 MB output
write, which is the roofline for this memory-regime problem.
"""

import numpy as np
from contextlib import ExitStack

from concourse import bass, tile, mybir
from concourse.tile import add_dep_helper
from concourse.bass_utils import run_bass_kernel_spmd

# Problem shapes (hardcoded per the task contract).
C, H, W = 256, 512, 512
N = 20
N_CORES = 8
HS = H // N_CORES          # 64 rows of the output per core
HB = 16                    # h-rows staged per output DMA chunk (8 DMAs
                           # total — Tile has 8 HWDGE sem lanes; a 9th DMA
                           # would reuse a lane and need an extra wait)
F32 = mybir.dt.float32
F32R = mybir.dt.float32r

VOXEL = (0.4, 0.4, 4.0)
LIDAR_RANGE = (-102.4, -102.4, -3.0, 102.4, 102.4, 1.0)
DOWNSAMPLE = 1

# Moving/stationary matmul dtype: "fp32r" (full-rate PE) or "fp32" (4 cyc/row).
MM_MODE = "fp32r"

_PROG = None          # cached Bass program
LAST_RESULTS = None   # BassKernelResults of the most recent run (for test.py)


def _host_factors(pred_box_infra, infra_features):
    """Per-box scalars, bilinear-sampled box features and separable gaussian
    profiles — all tiny. Coordinate math in float32 to match the reference
    bit-for-bit where it matters (floor/clip decisions)."""
    boxes = pred_box_infra[:N].astype(np.float32)
    feat = infra_features[0]                      # [C,H,W] float32
    l_corner = boxes.min(axis=1)                  # [N,3]
    r_corner = boxes.max(axis=1)
    sx = np.float32(VOXEL[0] * DOWNSAMPLE)
    sy = np.float32(VOXEL[1] * DOWNSAMPLE)
    x1 = (l_corner[:, 0] - np.float32(LIDAR_RANGE[0])) / sx
    y1 = (l_corner[:, 1] - np.float32(LIDAR_RANGE[1])) / sy
    x2 = (r_corner[:, 0] - np.float32(LIDAR_RANGE[0])) / sx
    y2 = (r_corner[:, 1] - np.float32(LIDAR_RANGE[1])) / sy
    bev_size = (y2 - y1) * (x2 - x1)              # [N]
    cx = np.float32(0.5) * (x1 + x2)
    cy = np.float32(0.5) * (y1 + y2)

    # bilinear sample at (cy, cx), matching the reference's clip/floor
    y = np.clip(cy, 0.0, H - 1.0).astype(np.float32)
    x = np.clip(cx, 0.0, W - 1.0).astype(np.float32)
    yl = np.floor(y).astype(np.int32)
    xl = np.floor(x).astype(np.int32)
    yh = np.minimum(yl + 1, H - 1)
    xh = np.minimum(xl + 1, W - 1)
    ly = (y - yl).astype(np.float64)[None, :]     # [1,N]
    lx = (x - xl).astype(np.float64)[None, :]
    g = lambda yi, xi: feat[:, yi, xi].astype(np.float64)   # [C,N]
    box_feats = (g(yl, xl) * (1 - ly) * (1 - lx)
                 + g(yl, xh) * (1 - ly) * lx
                 + g(yh, xl) * ly * (1 - lx)
                 + g(yh, xh) * ly * lx)           # [C,N] float64

    denom = 2.0 * bev_size.astype(np.float64) ** 2          # [N]
    hh = np.arange(H, dtype=np.float64)
    ww = np.arange(W, dtype=np.float64)
    gy = np.exp(-((hh[None, :] - x1.astype(np.float64)[:, None]) ** 2) / denom[:, None])
    gx = np.exp(-(ww[None, :] ** 2) / denom[:, None])

    a_t = np.ascontiguousarray((box_feats / N).T.astype(np.float32))  # [N,C]
    return a_t, gy.astype(np.float32), gx.astype(np.float32)


def _build_program():
    nc = bass.Bass("TRN2", target_bir_lowering=False, debug=False,
                   num_devices=N_CORES)
    # params = concat([a_t [N,C], gy [N,HS], gx [N,W]], axis=1): one DMA,
    # one semaphore (several input DMAs overflow the per-instruction
    # sync-wait budget of the first consumer).
    PF = C + HS + W
    params = nc.dram_tensor("params", [N, PF], F32, kind="ExternalInput").ap()
    out = nc.dram_tensor("out", [C, HS, W], F32, kind="ExternalOutput").ap()

    mm_dt = F32R if MM_MODE == "fp32r" else F32

    with ExitStack() as ctx:
        tc = ctx.enter_context(tile.TileContext(nc))
        const = ctx.enter_context(tc.tile_pool(name="const", bufs=1))
        # Deep pool: recycled slots' consumers are many iterations old by
        # reuse time, so Tile elides waits — TensorScalarPtr only has one
        # ISA sync-wait slot.
        bpool = ctx.enter_context(tc.tile_pool(name="bh", bufs=32))
        spool = ctx.enter_context(tc.tile_pool(name="stage", bufs=4))
        ppool = ctx.enter_context(tc.tile_pool(name="psum", bufs=8, space="PSUM"))

        # SWDGE for the input load: keeps all 8 HWDGE sem lanes free for
        # the 8 output DMAs (a 9th HWDGE user would need an extra wait).
        p_sb = const.tile([N, PF], F32)
        in_dma = nc.gpsimd.dma_start(p_sb[:], params[:])
        a_sb = p_sb[:, 0:C]
        gy_sb = p_sb[:, C:C + HS]
        gx_sb = p_sb[:, C + HS:PF]
        # fp32r matmul operands must be produced as fp32r (pre-rounded);
        # re-emit gx through the DVE into an fp32r tile.
        gx_mm = const.tile([N, W], mm_dt)
        nc.vector.tensor_copy(gx_mm[:], gx_sb)

        SW = HB * W
        SBUFS = 4                 # spool bufs (slot reuse period)
        NCHUNK = HS // HB
        # DVE ISA structs hold a single sync wait, but a recycled stage
        # slot needs {prev out-DMA done, prev-gen DVE copies done}. Before
        # each chunk's copies, two scratch memsets on the DVE each carry
        # ONE explicit wait; Tile's observed-tick subsumption then lets
        # the copies keep just their PE wait.
        scratch = const.tile([128, 4 * NCHUNK], F32)
        last_copy = {}            # slot -> last DVE copy (mybir inst)
        last_dma = {}             # slot -> out-DMA (mybir inst)
        col = [0]

        def touch_after(dep_inst):
            t = nc.vector.memset(scratch[:, col[0]:col[0] + 1], 0.0)
            col[0] += 1
            add_dep_helper(t.ins, dep_inst, sync=True,
                           reason="pre-cover stage slot release")
            return t

        tail_deps = []            # everything the tail drain must observe
        for hb in range(NCHUNK):
            stages = []
            for which in (0, 1):
                st = spool.tile([128, SW], F32, tag="stage")
                slot = (2 * hb + which) % SBUFS
                touches = []
                if slot in last_dma:
                    touches.append(touch_after(last_dma[slot]))
                    touches.append(touch_after(last_copy[slot]))
                stages.append((which, st, slot, touches))
            for hl in range(HB):
                h = hb * HB + hl
                b = bpool.tile([N, C], mm_dt)
                nc.vector.tensor_scalar_mul(b[:], a_sb, gy_sb[:, h:h + 1])
                for which, stage, slot, touches in stages:
                    ps = ppool.tile([128, W], F32)
                    mm = nc.tensor.matmul(
                        ps[:],
                        b[:, which * 128:(which + 1) * 128],
                        gx_mm[:],
                        start=True, stop=True,
                    )
                    cp = nc.vector.tensor_copy(
                        stage[:, hl * W:(hl + 1) * W], ps[:])
                    if hl == 0:
                        for t in touches:
                            add_dep_helper(cp.ins, t.ins, sync=False,
                                           reason="copies after slot touch")
                    last_copy[slot] = cp.ins
            for which, stage, slot, touches in stages:
                dma = nc.sync.dma_start(
                    out[which * 128:(which + 1) * 128,
                        hb * HB:(hb + 1) * HB, :],
                    stage[:].rearrange("p (h w) -> p h w", h=HB),
                )
                last_dma[slot] = dma.ins
                tail_deps.append(dma.ins)

        # The tail drain (SP) would otherwise carry one wait per
        # outstanding sem (8 DMA lanes + input DMA + PE + DVE) — its ISA
        # budget is one. Pre-cover every sem with single-wait SP nops;
        # add_sem_waits then elides them all on the drain.
        tail_deps = [mm.ins, cp.ins, in_dma.ins] + tail_deps
        for dep in tail_deps:
            tnop = nc.sync.nop(nofuse=True)
            add_dep_helper(tnop.ins, dep, sync=True,
                           reason="tail drain pre-cover")
    return nc


def _program():
    global _PROG
    if _PROG is None:
        _PROG = _build_program()
    return _PROG


def make_in_maps(pred_box_infra, infra_features):
    a_t, gy_full, gx = _host_factors(
        np.asarray(pred_box_infra, dtype=np.float32),
        np.asarray(infra_features, dtype=np.float32),
    )
    return [
        {
            "params": np.ascontiguousarray(np.concatenate(
                [a_t, gy_full[:, c * HS:(c + 1) * HS], gx], axis=1)),
        }
        for c in range(N_CORES)
    ]


def kernel(pred_box_infra, infra_features):
    global LAST_RESULTS
    in_maps = make_in_maps(pred_box_infra, infra_features)
    nc = _program()
    res = run_bass_kernel_spmd(nc, in_maps, core_ids=list(range(N_CORES)))
    LAST_RESULTS = res
    full = np.empty((1, C, H, W), dtype=np.float32)
    for c in range(N_CORES):
        full[0, :, c * HS:(c + 1) * HS, :] = res.results[c]["out"]
    return full

